# revision 19
# baseline (speedup 1.0000x reference)
"""Mamba mixer Bass kernel for 8 Trainium2 NeuronCores — v2.

Tensor-parallel over intermediate_size (4096 -> 512 channels per core).
v2 vs v1: activations stay in SBUF (no DRAM round trips), bf16 on the
DVE hot loop (2x tensor_tensor), dBu offloaded to GpSimd, exp(A*delta)
on the Scalar engine, phases interleaved so gate/out_proj matmuls and
the AllReduce overlap the selective scan.

Per-core layout: [channel partitions, (b, t) free]; hidden_states
pre-transposed on host to [H, B*L] bf16.
"""

import numpy as np
import ml_dtypes

import concourse.bass as bass
import concourse.mybir as mybir
import concourse.tile as tile
from concourse.bass_utils import run_bass_kernel_spmd

F32 = mybir.dt.float32
F32R = mybir.dt.float32r
BF16 = mybir.dt.bfloat16
AF = mybir.ActivationFunctionType
OP = mybir.AluOpType
BF = ml_dtypes.bfloat16

N_CORES = 8

CFG_FULL = dict(
    H=2048,      # hidden size
    IL=512,      # local intermediate channels (4096 / 8)
    N=16,        # ssm state size
    R=128,       # dt rank
    B=2,         # batch
    L=2048,      # sequence length
    NB=512,      # bl-chunk for matmul moving dim
)


def _split_sync_waits(nc, maxw=1):
    """walrus in this container accepts a single sem-wait per instruction;
    move extra waits onto preceding same-engine drains."""
    cnt = 0
    for bb in nc.main_func.blocks:
        insts = bb.instructions
        i = 0
        while i < len(insts):
            ins = insts[i]
            si = getattr(ins, "sync_info", None)
            waits = list(si.on_wait) if si is not None and si.on_wait else []
            if len(waits) > maxw:
                extra, keep = waits[:-maxw], waits[-maxw:]
                si.on_wait = keep
                pre = []
                for j in range(0, len(extra), maxw):
                    nop = mybir.InstDrain(
                        name=f"{ins.name}-wsplit-{j}", engine=ins.engine)
                    nop.sync_info = mybir.SyncInfo(
                        on_wait=extra[j:j + maxw], on_update=[])
                    pre.append(nop)
                insts[i:i] = pre
                i += len(pre)
                cnt += len(pre)
            i += 1
    return cnt


def build_program(cfg):
    H, IL, N, R, B, L = cfg["H"], cfg["IL"], cfg["N"], cfg["R"], cfg["B"], cfg["L"]
    NB = cfg["NB"]
    BL = B * L
    KH = H // 128          # k-tiles of the hidden contraction
    MD = IL // 128         # d-tiles (partition tiles of local channels)
    NBC = BL // NB         # bl chunks
    NBB = L // NB          # chunks per sequence
    KCONV = 4

    nc = bass.Bass()

    hsT = nc.declare_dram_parameter("hsT", [H, BL], BF16, isOutput=False)
    winT = nc.declare_dram_parameter("winT", [H, 2 * IL], BF16, isOutput=False)
    convw = nc.declare_dram_parameter("convw", [IL, KCONV], F32, isOutput=False)
    convb = nc.declare_dram_parameter("convb", [IL, 1], F32, isOutput=False)
    xwT = nc.declare_dram_parameter("xwT", [IL, R + 2 * N], BF16, isOutput=False)
    dtwT = nc.declare_dram_parameter("dtwT", [R, IL], F32, isOutput=False)
    dtb = nc.declare_dram_parameter("dtb", [IL, 1], F32, isOutput=False)
    Amat = nc.declare_dram_parameter("Amat", [IL, N], F32, isOutput=False)
    Dp = nc.declare_dram_parameter("Dp", [IL, 1], F32, isOutput=False)
    woT = nc.declare_dram_parameter("woT", [IL, H], BF16, isOutput=False)
    out_part = nc.declare_dram_parameter("out_part", [H, BL], BF16, isOutput=True)

    ssm_part = nc.dram_tensor("ssm_part", [R + 2 * N, BL], F32)
    ssm_full = nc.dram_tensor("ssm_full", [R + 2 * N, BL], F32, addr_space="Shared")
    bc16_d = nc.dram_tensor("bc16_d", [2 * N, BL], BF16)
    gate_d = nc.dram_tensor("gate_d", [IL, BL], BF16)

    with tile.TileContext(nc) as tc:
        # pools with hand-managed lifetimes (dpool must open after the
        # phase-A pools close but outlive upool)
        cp_cm = tc.tile_pool(name="const", bufs=1); cp = cp_cm.__enter__()
        up_cm = tc.tile_pool(name="upool", bufs=1); up = up_cm.__enter__()

        A_t, cw_t, cb_t, db_t, D_t = [], [], [], [], []
        for dt in range(MD):
            rows = slice(dt * 128, (dt + 1) * 128)
            a = cp.tile([128, N], F32, name=f"A{dt}", tag=f"A{dt}")
            nc.sync.dma_start(a[:], Amat[rows, :])
            A_t.append(a)
            w = cp.tile([128, KCONV], F32, name=f"cw{dt}", tag=f"cw{dt}")
            nc.sync.dma_start(w[:], convw[rows, :])
            cw_t.append(w)
            bb_ = cp.tile([128, 1], F32, name=f"cb{dt}", tag=f"cb{dt}")
            nc.sync.dma_start(bb_[:], convb[rows, :])
            cb_t.append(bb_)
            d_ = cp.tile([128, 1], F32, name=f"db{dt}", tag=f"db{dt}")
            nc.sync.dma_start(d_[:], dtb[rows, :])
            db_t.append(d_)
            dd = cp.tile([128, 1], F32, name=f"Dp{dt}", tag=f"Dp{dt}")
            nc.sync.dma_start(dd[:], Dp[rows, :])
            D_t.append(dd)

        u_t = {(b_, dt): up.tile([128, L], BF16, name=f"u{b_}_{dt}",
                                 tag=f"u{b_}_{dt}")
               for b_ in range(B) for dt in range(MD)}

        # ---- phase A: in_proj (x first), conv+silu -> u; x_proj partials
        #      trailing one chunk; gate matmuls issued after the AllReduce ----
        pa_cms = [tc.tile_pool(name="wA", bufs=1),
                  tc.tile_pool(name="hst", bufs=2),
                  tc.tile_pool(name="xA", bufs=2),
                  tc.tile_pool(name="psA", bufs=4, space="PSUM"),
                  tc.tile_pool(name="stA", bufs=4),
                  tc.tile_pool(name="xwB", bufs=1),
                  tc.tile_pool(name="psB", bufs=2, space="PSUM"),
                  tc.tile_pool(name="stB", bufs=4)]
        wp, hp, xp, pp, sp, xwp, pbp, sbp = [cm.__enter__() for cm in pa_cms]

        w_tiles = {}
        for m in range(2 * MD):
            for k in range(KH):
                wt = wp.tile([128, 128], BF16, name=f"w{m}_{k}", tag=f"w{m}_{k}")
                nc.sync.dma_start(
                    wt[:], winT[k * 128:(k + 1) * 128, m * 128:(m + 1) * 128])
                w_tiles[(m, k)] = wt
        xw_a, xw_b = [], []
        for dt in range(MD):
            rows = slice(dt * 128, (dt + 1) * 128)
            ta = xwp.tile([128, R], BF16, name=f"xwa{dt}", tag=f"xwa{dt}")
            nc.sync.dma_start(ta[:], xwT[rows, 0:R])
            xw_a.append(ta)
            tb = xwp.tile([128, 2 * N], BF16, name=f"xwb{dt}", tag=f"xwb{dt}")
            nc.sync.dma_start(tb[:], xwT[rows, R:R + 2 * N])
            xw_b.append(tb)

        def xproj_chunk(nb):
            csl = slice(nb * NB, (nb + 1) * NB)
            bb_, cloc = nb // NBB, slice((nb % NBB) * NB, (nb % NBB + 1) * NB)
            psa = pbp.tile([R, NB], F32, name="psBa", tag="psBa")
            psb = pbp.tile([2 * N, NB], F32, name="psBb", tag="psBb")
            for dt in range(MD):
                nc.tensor.matmul(psa[:], xw_a[dt][:], u_t[(bb_, dt)][:, cloc],
                                 start=(dt == 0), stop=(dt == MD - 1))
            for dt in range(MD):
                nc.tensor.matmul(psb[:], xw_b[dt][:], u_t[(bb_, dt)][:, cloc],
                                 start=(dt == 0), stop=(dt == MD - 1))
            sta = sbp.tile([R, NB], F32, name="stBa", tag="stBa")
            nc.scalar.copy(sta[:], psa[:])
            nc.sync.dma_start(ssm_part[0:R, csl], sta[:])
            stb = sbp.tile([2 * N, NB], F32, name="stBb", tag="stBb")
            nc.scalar.copy(stb[:], psb[:])
            nc.sync.dma_start(ssm_part[R:R + 2 * N, csl], stb[:])

        x_prev = [None] * MD
        for nb in range(NBC):
            csl = slice(nb * NB, (nb + 1) * NB)
            hst = []
            for k in range(KH):
                ht = hp.tile([128, NB], BF16, name=f"hst{k}", tag=f"hst{k}")
                nc.sync.dma_start(ht[:], hsT[k * 128:(k + 1) * 128, csl])
                hst.append(ht)
            for m in range(MD):
                ps = pp.tile([128, NB], F32, name="psA", tag="psA")
                for k in range(KH):
                    nc.tensor.matmul(ps[:], w_tiles[(m, k)][:], hst[k][:],
                                     start=(k == 0), stop=(k == KH - 1))
                xc = xp.tile([128, NB], BF16, name=f"x{m}", tag=f"x{m}", bufs=2)
                nc.scalar.copy(xc[:], ps[:])
                tmp = sp.tile([128, NB], BF16, name="ctmp", tag="ctmp", bufs=3)
                nc.vector.tensor_scalar_mul(
                    tmp[:], xc[:], cw_t[m][:, KCONV - 1:KCONV])
                for s in range(1, KCONV):
                    nc.vector.scalar_tensor_tensor(
                        tmp[:, s:], xc[:, :NB - s],
                        cw_t[m][:, KCONV - 1 - s:KCONV - s],
                        tmp[:, s:], OP.mult, OP.add)
                if nb % NBB != 0:
                    for s in range(1, KCONV):
                        nc.vector.scalar_tensor_tensor(
                            tmp[:, 0:s], x_prev[m][:, NB - s:NB],
                            cw_t[m][:, KCONV - 1 - s:KCONV - s],
                            tmp[:, 0:s], OP.mult, OP.add)
                nc.scalar.activation(
                    u_t[(nb // NBB, m)][:, slice((nb % NBB) * NB,
                                                 (nb % NBB + 1) * NB)],
                    tmp[:], AF.Silu, bias=cb_t[m][:, 0:1])
                x_prev[m] = xc
            if nb > 0:
                xproj_chunk(nb - 1)
        xproj_chunk(NBC - 1)

        nc.gpsimd.collective_compute(
            "AllReduce", OP.add,
            replica_groups=[list(range(N_CORES))],
            ins=[ssm_part[:, :]],
            outs=[ssm_full[:, :]],
        )

        # gate matmuls + silu (PE/ACT fill while AllReduce runs)
        for nb in range(NBC):
            csl = slice(nb * NB, (nb + 1) * NB)
            hst = []
            for k in range(KH):
                ht = hp.tile([128, NB], BF16, name=f"hst{k}", tag=f"hst{k}")
                nc.sync.dma_start(ht[:], hsT[k * 128:(k + 1) * 128, csl])
                hst.append(ht)
            for m in range(MD):
                ps = pp.tile([128, NB], F32, name="psA", tag="psA")
                for k in range(KH):
                    nc.tensor.matmul(ps[:], w_tiles[(MD + m, k)][:], hst[k][:],
                                     start=(k == 0), stop=(k == KH - 1))
                stg = sp.tile([128, NB], BF16, name="stG", tag="stG", bufs=3)
                nc.scalar.activation(stg[:], ps[:], AF.Silu)
                nc.sync.dma_start(gate_d[m * 128:(m + 1) * 128, csl], stg[:])

        for cm in reversed(pa_cms):
            cm.__exit__(None, None, None)

        # ---- phase C-prep: dt_proj -> delta (bf16, SBUF), du, y-init;
        #      B/C rows downcast to bf16 and staged in DRAM ----
        dp_cm = tc.tile_pool(name="dpool", bufs=1); dp = dp_cm.__enter__()
        dl_t = {(b_, dt): dp.tile([128, L], BF16, name=f"dl{b_}_{dt}",
                                  tag=f"dl{b_}_{dt}")
                for b_ in range(B) for dt in range(MD)}
        du_t = {(b_, dt): dp.tile([128, L], BF16, name=f"du{b_}_{dt}",
                                  tag=f"du{b_}_{dt}")
                for b_ in range(B) for dt in range(MD)}
        y32 = {}   # per-dt fp32 accumulators, reused across batches

        pc_cms = [tc.tile_pool(name="dtwC", bufs=1),
                  tc.tile_pool(name="dtlr", bufs=2),
                  tc.tile_pool(name="psC", bufs=4, space="PSUM"),
                  tc.tile_pool(name="stC", bufs=4),
                  tc.tile_pool(name="bcC", bufs=2)]
        dwp, lrp, pcp, scp, bcp = [cm.__enter__() for cm in pc_cms]

        dtw_t = []
        for dt in range(MD):
            t_ = dwp.tile([R, 128], F32R, name=f"dtw{dt}", tag=f"dtw{dt}")
            nc.sync.dma_start(
                t_[:], dtwT[:, dt * 128:(dt + 1) * 128].bitcast(F32R))
            dtw_t.append(t_)
        # B/C downcast: [2N, BL] f32 -> bf16 -> DRAM, in 4 column chunks
        QBC = BL // 4
        for q in range(4):
            qsl = slice(q * QBC, (q + 1) * QBC)
            bc32 = bcp.tile([2 * N, QBC], F32, name="bc32", tag="bc32")
            nc.sync.dma_start(bc32[:], ssm_full[R:R + 2 * N, qsl])
            bc16 = bcp.tile([2 * N, QBC], BF16, name="bc16", tag="bc16")
            nc.scalar.copy(bc16[:], bc32[:])
            nc.sync.dma_start(bc16_d[:, qsl], bc16[:])
        for nb in range(NBC):
            csl = slice(nb * NB, (nb + 1) * NB)
            bb_, cloc = nb // NBB, slice((nb % NBB) * NB, (nb % NBB + 1) * NB)
            lr = lrp.tile([R, NB], F32R, name="dtlr", tag="dtlr")
            nc.sync.dma_start(lr[:], ssm_full[0:R, csl].bitcast(F32R))
            for dt in range(MD):
                ps = pcp.tile([128, NB], F32, name="psC", tag="psC")
                nc.tensor.matmul(ps[:], dtw_t[dt][:], lr[:],
                                 start=True, stop=True)
                # softplus(x) = ln(1 + exp(x))
                ex = scp.tile([128, NB], BF16, name="ex", tag="ex")
                nc.scalar.activation(ex[:], ps[:], AF.Exp,
                                     bias=db_t[dt][:, 0:1])
                nc.scalar.activation(dl_t[(bb_, dt)][:, cloc], ex[:],
                                     AF.Ln, bias=1.0)
                nc.vector.tensor_tensor(du_t[(bb_, dt)][:, cloc],
                                        dl_t[(bb_, dt)][:, cloc],
                                        u_t[(bb_, dt)][:, cloc], OP.mult)


        for cm in reversed(pc_cms):
            cm.__exit__(None, None, None)

        # ---- phase C-scan + gating + out_proj, per batch ----
        ps_cms = [tc.tile_pool(name="woD", bufs=1),
                  tc.tile_pool(name="bcS", bufs=2),
                  tc.tile_pool(name="bcB", bufs=1),
                  tc.tile_pool(name="dAS", bufs=2),
                  tc.tile_pool(name="dBS", bufs=2),
                  tc.tile_pool(name="hS", bufs=1),
                  tc.tile_pool(name="hcS", bufs=1),
                  tc.tile_pool(name="gbS", bufs=1),
                  tc.tile_pool(name="psD", bufs=4, space="PSUM"),
                  tc.tile_pool(name="psX", bufs=4, space="PSUM"),
                  tc.tile_pool(name="stD", bufs=2)]
        wop, bsp, bbp, dap, dbp, hsp, hcp, gbp, pdp, pxp, sdp = \
            [cm.__enter__() for cm in ps_cms]

        wo_t = {}
        for kk in range(MD):
            for m in range(KH):
                wt = wop.tile([128, 128], BF16, name=f"wo{kk}_{m}",
                              tag=f"wo{kk}_{m}")
                nc.sync.dma_start(
                    wt[:], woT[kk * 128:(kk + 1) * 128, m * 128:(m + 1) * 128])
                wo_t[(kk, m)] = wt
        ones16 = wop.tile([1, 128], BF16, name="ones16", tag="ones16")
        nc.vector.memset(ones16[:], 1.0)

        NQ = L // NB
        seq = [(b, n) for b in range(B) for n in range(N)]
        prod = {}

        def produce(b, n):
            # replicate B/C rows across partitions via PE outer product;
            # rows staged at partition 0 (matmul base-partition constraint)
            rowB = bbp.tile([1, L], BF16, name="rowB", tag="rowB", bufs=1)
            nc.sync.dma_start(rowB[:], bc16_d[n:n + 1, b * L:(b + 1) * L])
            rowC = bbp.tile([1, L], BF16, name="rowC", tag="rowC", bufs=1)
            nc.sync.dma_start(rowC[:], bc16_d[N + n:N + n + 1,
                                              b * L:(b + 1) * L])
            Bt = bsp.tile([128, L], BF16, name="Bt", tag="Bt")
            Ct = bsp.tile([128, L], BF16, name="Ct", tag="Ct")
            for q in range(NQ):
                qsl = slice(q * NB, (q + 1) * NB)
                psq = pxp.tile([128, NB], F32, name="psX", tag="psX")
                nc.tensor.matmul(psq[:], ones16[:], rowB[:, qsl],
                                 start=True, stop=True)
                nc.scalar.copy(Bt[:, qsl], psq[:])
            for q in range(NQ):
                qsl = slice(q * NB, (q + 1) * NB)
                psq = pxp.tile([128, NB], F32, name="psX", tag="psX")
                nc.tensor.matmul(psq[:], ones16[:], rowC[:, qsl],
                                 start=True, stop=True)
                nc.scalar.copy(Ct[:, qsl], psq[:])
            return Bt, Ct

        def ensure(i):
            if 0 <= i < len(seq) and i not in prod:
                prod[i] = produce(*seq[i])

        ensure(0)
        ensure(1)
        for i, (b, n) in enumerate(seq):
            Bt, Ct = prod[i]
            bsl = slice(b * L, (b + 1) * L)
            if n == 0:
                # fresh fp32 accumulators for this batch (tag reuse across b)
                for dt in range(MD):
                    y32[dt] = dp.tile([128, L], F32, name=f"y32_{dt}",
                                      tag=f"y32_{dt}")
                    nc.vector.tensor_scalar_mul(
                        y32[dt][:], u_t[(b, dt)][:], D_t[dt][:, 0:1])
            if n == N - 1:
                gbs = {}
                for dt in range(2):
                    gb = gbp.tile([128, L], BF16, name="gb", tag="gb", bufs=2)
                    nc.sync.dma_start(
                        gb[:], gate_d[dt * 128:(dt + 1) * 128, bsl])
                    gbs[dt] = gb
            for dt in range(MD):
                dA = dap.tile([128, L], BF16, name="dA", tag="dA")
                nc.scalar.activation(dA[:], dl_t[(b, dt)][:], AF.Exp,
                                     scale=A_t[dt][:, n:n + 1])
                dBu = dbp.tile([128, L], BF16, name="dBu", tag="dBu")
                nc.gpsimd.tensor_tensor(dBu[:], du_t[(b, dt)][:], Bt[:],
                                        OP.mult)
                h = hsp.tile([128, L], BF16, name="h", tag="h")
                nc.vector.tensor_tensor_scan(h[:], dA[:], dBu[:], 0.0,
                                             op0=OP.mult, op1=OP.add)
                hc = hcp.tile([128, L], F32, name="hc", tag="hc")
                nc.vector.tensor_tensor(hc[:], h[:], Ct[:], OP.mult)
                nc.vector.tensor_tensor(y32[dt][:], y32[dt][:], hc[:],
                                        OP.add)
            ensure(i + 2)
            if n != N - 1:
                continue
            # gating for this batch: write into the dead u tiles (bf16)
            ygs = []
            for dt in range(MD):
                if dt not in gbs:
                    gb = gbp.tile([128, L], BF16, name="gb", tag="gb", bufs=2)
                    nc.sync.dma_start(
                        gb[:], gate_d[dt * 128:(dt + 1) * 128, bsl])
                    gbs[dt] = gb
                nc.vector.tensor_tensor(u_t[(b, dt)][:], y32[dt][:],
                                        gbs[dt][:], OP.mult)
                ygs.append(u_t[(b, dt)])
            for m in range(KH):
                for cc in range(L // NB):
                    csl = slice(cc * NB, (cc + 1) * NB)
                    osl = slice(b * L + cc * NB, b * L + (cc + 1) * NB)
                    ps = pdp.tile([128, NB], F32, name="psD", tag="psD")
                    for kk in range(MD):
                        nc.tensor.matmul(ps[:], wo_t[(kk, m)][:],
                                         ygs[kk][:, csl],
                                         start=(kk == 0), stop=(kk == MD - 1))
                    st = sdp.tile([128, NB], BF16, name="stD", tag="stD")
                    nc.scalar.copy(st[:], ps[:])
                    nc.sync.dma_start(
                        out_part[m * 128:(m + 1) * 128, osl], st[:])

        for cm in reversed(ps_cms):
            cm.__exit__(None, None, None)
        dp_cm.__exit__(None, None, None)
        up_cm.__exit__(None, None, None)
        cp_cm.__exit__(None, None, None)

    _split_sync_waits(nc)
    return nc


def _bf(x):
    return np.ascontiguousarray(np.asarray(x, dtype=np.float32).astype(BF))


def make_in_maps(cfg, hidden_states, in_proj_w, conv_w, conv_b, x_proj_w,
                 dt_proj_w, dt_proj_b, A_log, D_param, out_proj_w):
    H, IL, N, R, B, L = cfg["H"], cfg["IL"], cfg["N"], cfg["R"], cfg["B"], cfg["L"]
    BL = B * L
    I_full = IL * N_CORES
    c = np.ascontiguousarray
    hsT = _bf(np.asarray(hidden_states, np.float32).reshape(BL, H).T)
    A_full = -np.exp(np.asarray(A_log, np.float32))
    in_maps = []
    for ci in range(N_CORES):
        sl = slice(ci * IL, (ci + 1) * IL)
        gsl = slice(I_full + ci * IL, I_full + (ci + 1) * IL)
        wxT = np.asarray(in_proj_w, np.float32)[sl, :].T
        wgT = np.asarray(in_proj_w, np.float32)[gsl, :].T
        in_maps.append({
            "hsT": hsT,
            "winT": _bf(np.concatenate([wxT, wgT], axis=1)),
            "convw": c(np.asarray(conv_w, np.float32)[sl, 0, :]),
            "convb": c(np.asarray(conv_b, np.float32)[sl].reshape(IL, 1)),
            "xwT": _bf(np.asarray(x_proj_w, np.float32)[:, sl].T),
            "dtwT": c(np.asarray(dt_proj_w, np.float32)[sl, :].T),
            "dtb": c(np.asarray(dt_proj_b, np.float32)[sl].reshape(IL, 1)),
            "Amat": c(A_full[sl, :]),
            "Dp": c(np.asarray(D_param, np.float32)[sl].reshape(IL, 1)),
            "woT": _bf(np.asarray(out_proj_w, np.float32)[:, sl].T),
        })
    return in_maps


_PROG_CACHE = {}


def run(cfg, inputs, **run_kwargs):
    key = tuple(sorted(cfg.items()))
    if key not in _PROG_CACHE:
        _PROG_CACHE[key] = build_program(cfg)
    nc = _PROG_CACHE[key]
    in_maps = make_in_maps(cfg, **inputs)
    res = run_bass_kernel_spmd(nc, in_maps, list(range(N_CORES)), **run_kwargs)
    H, B, L = cfg["H"], cfg["B"], cfg["L"]
    out = np.zeros((H, B * L), np.float64)
    for ci in range(N_CORES):
        out += np.asarray(res.results[ci]["out_part"], np.float32)
    full = out.astype(np.float32).T.reshape(B, L, H)
    return full, res


def kernel(**inputs):
    out, _ = run(CFG_FULL, inputs)
    return out


# revision 21
# speedup vs baseline: 1.1037x; 1.1037x over previous
"""Mamba mixer Bass kernel for 8 Trainium2 NeuronCores — v2.

Tensor-parallel over intermediate_size (4096 -> 512 channels per core).
v2 vs v1: activations stay in SBUF (no DRAM round trips), bf16 on the
DVE hot loop (2x tensor_tensor), dBu offloaded to GpSimd, exp(A*delta)
on the Scalar engine, phases interleaved so gate/out_proj matmuls and
the AllReduce overlap the selective scan.

Per-core layout: [channel partitions, (b, t) free]; hidden_states
pre-transposed on host to [H, B*L] bf16.
"""

import numpy as np
import ml_dtypes

import concourse.bass as bass
import concourse.mybir as mybir
import concourse.tile as tile
from concourse.bass_utils import run_bass_kernel_spmd

F32 = mybir.dt.float32
F32R = mybir.dt.float32r
BF16 = mybir.dt.bfloat16
AF = mybir.ActivationFunctionType
OP = mybir.AluOpType
BF = ml_dtypes.bfloat16

N_CORES = 8

CFG_FULL = dict(
    H=2048,      # hidden size
    IL=512,      # local intermediate channels (4096 / 8)
    N=16,        # ssm state size
    R=128,       # dt rank
    B=2,         # batch
    L=2048,      # sequence length
    NB=512,      # bl-chunk for matmul moving dim
)


def _split_sync_waits(nc, maxw=1):
    """walrus in this container accepts a single sem-wait per instruction;
    move extra waits onto preceding same-engine drains."""
    cnt = 0
    for bb in nc.main_func.blocks:
        insts = bb.instructions
        i = 0
        while i < len(insts):
            ins = insts[i]
            si = getattr(ins, "sync_info", None)
            waits = list(si.on_wait) if si is not None and si.on_wait else []
            if len(waits) > maxw:
                extra, keep = waits[:-maxw], waits[-maxw:]
                si.on_wait = keep
                pre = []
                for j in range(0, len(extra), maxw):
                    nop = mybir.InstDrain(
                        name=f"{ins.name}-wsplit-{j}", engine=ins.engine)
                    nop.sync_info = mybir.SyncInfo(
                        on_wait=extra[j:j + maxw], on_update=[])
                    pre.append(nop)
                insts[i:i] = pre
                i += len(pre)
                cnt += len(pre)
            i += 1
    return cnt


def build_program(cfg):
    H, IL, N, R, B, L = cfg["H"], cfg["IL"], cfg["N"], cfg["R"], cfg["B"], cfg["L"]
    NB = cfg["NB"]
    BL = B * L
    KH = H // 128          # k-tiles of the hidden contraction
    MD = IL // 128         # d-tiles (partition tiles of local channels)
    NBC = BL // NB         # bl chunks
    NBB = L // NB          # chunks per sequence
    KCONV = 4

    nc = bass.Bass()

    hsT = nc.declare_dram_parameter("hsT", [H, BL], BF16, isOutput=False)
    winT = nc.declare_dram_parameter("winT", [H, 2 * IL], BF16, isOutput=False)
    convw = nc.declare_dram_parameter("convw", [IL, KCONV], F32, isOutput=False)
    convb = nc.declare_dram_parameter("convb", [IL, 1], F32, isOutput=False)
    xwT = nc.declare_dram_parameter("xwT", [IL, R + 2 * N], BF16, isOutput=False)
    dtwT = nc.declare_dram_parameter("dtwT", [R, IL], F32, isOutput=False)
    dtb = nc.declare_dram_parameter("dtb", [IL, 1], F32, isOutput=False)
    Amat = nc.declare_dram_parameter("Amat", [IL, N], F32, isOutput=False)
    Dp = nc.declare_dram_parameter("Dp", [IL, 1], F32, isOutput=False)
    woT = nc.declare_dram_parameter("woT", [IL, H], BF16, isOutput=False)
    out_part = nc.declare_dram_parameter("out_part", [H, BL], BF16, isOutput=True)

    ssm_part = nc.dram_tensor("ssm_part", [R + 2 * N, BL], F32)
    ssm_full = nc.dram_tensor("ssm_full", [R + 2 * N, BL], F32, addr_space="Shared")
    bc16_d = nc.dram_tensor("bc16_d", [2 * N, BL], BF16)
    gate_d = nc.dram_tensor("gate_d", [IL, BL], BF16)

    with tile.TileContext(nc) as tc:
        # pools with hand-managed lifetimes (dpool must open after the
        # phase-A pools close but outlive upool)
        cp_cm = tc.tile_pool(name="const", bufs=1); cp = cp_cm.__enter__()
        up_cm = tc.tile_pool(name="upool", bufs=1); up = up_cm.__enter__()

        A_t, cw_t, cb_t, db_t, D_t = [], [], [], [], []
        for dt in range(MD):
            rows = slice(dt * 128, (dt + 1) * 128)
            a = cp.tile([128, N], F32, name=f"A{dt}", tag=f"A{dt}")
            nc.sync.dma_start(a[:], Amat[rows, :])
            A_t.append(a)
            w = cp.tile([128, KCONV], F32, name=f"cw{dt}", tag=f"cw{dt}")
            nc.sync.dma_start(w[:], convw[rows, :])
            cw_t.append(w)
            bb_ = cp.tile([128, 1], F32, name=f"cb{dt}", tag=f"cb{dt}")
            nc.sync.dma_start(bb_[:], convb[rows, :])
            cb_t.append(bb_)
            d_ = cp.tile([128, 1], F32, name=f"db{dt}", tag=f"db{dt}")
            nc.sync.dma_start(d_[:], dtb[rows, :])
            db_t.append(d_)
            dd = cp.tile([128, 1], F32, name=f"Dp{dt}", tag=f"Dp{dt}")
            nc.sync.dma_start(dd[:], Dp[rows, :])
            D_t.append(dd)

        u_t = {(b_, dt): up.tile([128, L], BF16, name=f"u{b_}_{dt}",
                                 tag=f"u{b_}_{dt}")
               for b_ in range(B) for dt in range(MD)}

        # ---- phase A: in_proj (x first), conv+silu -> u; x_proj partials
        #      trailing one chunk; gate matmuls issued after the AllReduce ----
        pa_cms = [tc.tile_pool(name="wA", bufs=1),
                  tc.tile_pool(name="hst", bufs=2),
                  tc.tile_pool(name="xA", bufs=2),
                  tc.tile_pool(name="psA", bufs=4, space="PSUM"),
                  tc.tile_pool(name="stA", bufs=4),
                  tc.tile_pool(name="xwB", bufs=1),
                  tc.tile_pool(name="psB", bufs=2, space="PSUM"),
                  tc.tile_pool(name="stB", bufs=4)]
        wp, hp, xp, pp, sp, xwp, pbp, sbp = [cm.__enter__() for cm in pa_cms]

        w_tiles = {}
        for m in range(2 * MD):
            for k in range(KH):
                wt = wp.tile([128, 128], BF16, name=f"w{m}_{k}", tag=f"w{m}_{k}")
                nc.sync.dma_start(
                    wt[:], winT[k * 128:(k + 1) * 128, m * 128:(m + 1) * 128])
                w_tiles[(m, k)] = wt
        xw_a, xw_b = [], []
        for dt in range(MD):
            rows = slice(dt * 128, (dt + 1) * 128)
            ta = xwp.tile([128, R], BF16, name=f"xwa{dt}", tag=f"xwa{dt}")
            nc.sync.dma_start(ta[:], xwT[rows, 0:R])
            xw_a.append(ta)
            tb = xwp.tile([128, 2 * N], BF16, name=f"xwb{dt}", tag=f"xwb{dt}")
            nc.sync.dma_start(tb[:], xwT[rows, R:R + 2 * N])
            xw_b.append(tb)

        def xproj_chunk(nb):
            csl = slice(nb * NB, (nb + 1) * NB)
            bb_, cloc = nb // NBB, slice((nb % NBB) * NB, (nb % NBB + 1) * NB)
            psa = pbp.tile([R, NB], F32, name="psBa", tag="psBa")
            psb = pbp.tile([2 * N, NB], F32, name="psBb", tag="psBb")
            for dt in range(MD):
                nc.tensor.matmul(psa[:], xw_a[dt][:], u_t[(bb_, dt)][:, cloc],
                                 start=(dt == 0), stop=(dt == MD - 1))
            for dt in range(MD):
                nc.tensor.matmul(psb[:], xw_b[dt][:], u_t[(bb_, dt)][:, cloc],
                                 start=(dt == 0), stop=(dt == MD - 1))
            sta = sbp.tile([R, NB], F32, name="stBa", tag="stBa")
            nc.scalar.copy(sta[:], psa[:])
            nc.sync.dma_start(ssm_part[0:R, csl], sta[:])
            stb = sbp.tile([2 * N, NB], F32, name="stBb", tag="stBb")
            nc.scalar.copy(stb[:], psb[:])
            nc.sync.dma_start(ssm_part[R:R + 2 * N, csl], stb[:])

        x_prev = [None] * MD
        for nb in range(NBC):
            csl = slice(nb * NB, (nb + 1) * NB)
            hst = []
            for k in range(KH):
                ht = hp.tile([128, NB], BF16, name=f"hst{k}", tag=f"hst{k}")
                nc.sync.dma_start(ht[:], hsT[k * 128:(k + 1) * 128, csl])
                hst.append(ht)
            for m in range(MD):
                ps = pp.tile([128, NB], F32, name="psA", tag="psA")
                for k in range(KH):
                    nc.tensor.matmul(ps[:], w_tiles[(m, k)][:], hst[k][:],
                                     start=(k == 0), stop=(k == KH - 1))
                xc = xp.tile([128, NB], BF16, name=f"x{m}", tag=f"x{m}", bufs=2)
                nc.scalar.copy(xc[:], ps[:])
                tmp = sp.tile([128, NB], BF16, name="ctmp", tag="ctmp", bufs=3)
                nc.vector.tensor_scalar_mul(
                    tmp[:], xc[:], cw_t[m][:, KCONV - 1:KCONV])
                for s in range(1, KCONV):
                    nc.vector.scalar_tensor_tensor(
                        tmp[:, s:], xc[:, :NB - s],
                        cw_t[m][:, KCONV - 1 - s:KCONV - s],
                        tmp[:, s:], OP.mult, OP.add)
                if nb % NBB != 0:
                    for s in range(1, KCONV):
                        nc.vector.scalar_tensor_tensor(
                            tmp[:, 0:s], x_prev[m][:, NB - s:NB],
                            cw_t[m][:, KCONV - 1 - s:KCONV - s],
                            tmp[:, 0:s], OP.mult, OP.add)
                nc.scalar.activation(
                    u_t[(nb // NBB, m)][:, slice((nb % NBB) * NB,
                                                 (nb % NBB + 1) * NB)],
                    tmp[:], AF.Silu, bias=cb_t[m][:, 0:1])
                x_prev[m] = xc
            if nb > 0:
                xproj_chunk(nb - 1)
        xproj_chunk(NBC - 1)

        nc.gpsimd.collective_compute(
            "AllReduce", OP.add,
            replica_groups=[list(range(N_CORES))],
            ins=[ssm_part[:, :]],
            outs=[ssm_full[:, :]],
        )

        # gate matmuls + silu (PE/ACT fill while AllReduce runs)
        for nb in range(NBC):
            csl = slice(nb * NB, (nb + 1) * NB)
            hst = []
            for k in range(KH):
                ht = hp.tile([128, NB], BF16, name=f"hst{k}", tag=f"hst{k}")
                nc.sync.dma_start(ht[:], hsT[k * 128:(k + 1) * 128, csl])
                hst.append(ht)
            for m in range(MD):
                ps = pp.tile([128, NB], F32, name="psA", tag="psA")
                for k in range(KH):
                    nc.tensor.matmul(ps[:], w_tiles[(MD + m, k)][:], hst[k][:],
                                     start=(k == 0), stop=(k == KH - 1))
                stg = sp.tile([128, NB], BF16, name="stG", tag="stG", bufs=3)
                nc.scalar.activation(stg[:], ps[:], AF.Silu)
                nc.sync.dma_start(gate_d[m * 128:(m + 1) * 128, csl], stg[:])

        for cm in reversed(pa_cms):
            cm.__exit__(None, None, None)

        # ---- phase C-prep: dt_proj -> delta (bf16, SBUF), du, y-init;
        #      B/C rows downcast to bf16 and staged in DRAM ----
        dp_cm = tc.tile_pool(name="dpool", bufs=1); dp = dp_cm.__enter__()
        dl_t = {(b_, dt): dp.tile([128, L], BF16, name=f"dl{b_}_{dt}",
                                  tag=f"dl{b_}_{dt}")
                for b_ in range(B) for dt in range(MD)}
        du_t = {(b_, dt): dp.tile([128, L], BF16, name=f"du{b_}_{dt}",
                                  tag=f"du{b_}_{dt}")
                for b_ in range(B) for dt in range(MD)}
        y16 = {}   # per-dt bf16 ping-pong accumulators, reused across batches

        pc_cms = [tc.tile_pool(name="dtwC", bufs=1),
                  tc.tile_pool(name="dtlr", bufs=2),
                  tc.tile_pool(name="psC", bufs=4, space="PSUM"),
                  tc.tile_pool(name="stC", bufs=4),
                  tc.tile_pool(name="bcC", bufs=2)]
        dwp, lrp, pcp, scp, bcp = [cm.__enter__() for cm in pc_cms]

        dtw_t = []
        for dt in range(MD):
            t_ = dwp.tile([R, 128], F32R, name=f"dtw{dt}", tag=f"dtw{dt}")
            nc.sync.dma_start(
                t_[:], dtwT[:, dt * 128:(dt + 1) * 128].bitcast(F32R))
            dtw_t.append(t_)
        # B/C downcast: [2N, BL] f32 -> bf16 -> DRAM, in 4 column chunks
        QBC = BL // 4
        for q in range(4):
            qsl = slice(q * QBC, (q + 1) * QBC)
            bc32 = bcp.tile([2 * N, QBC], F32, name="bc32", tag="bc32")
            nc.sync.dma_start(bc32[:], ssm_full[R:R + 2 * N, qsl])
            bc16 = bcp.tile([2 * N, QBC], BF16, name="bc16", tag="bc16")
            nc.scalar.copy(bc16[:], bc32[:])
            nc.sync.dma_start(bc16_d[:, qsl], bc16[:])
        for nb in range(NBC):
            csl = slice(nb * NB, (nb + 1) * NB)
            bb_, cloc = nb // NBB, slice((nb % NBB) * NB, (nb % NBB + 1) * NB)
            lr = lrp.tile([R, NB], F32R, name="dtlr", tag="dtlr")
            nc.sync.dma_start(lr[:], ssm_full[0:R, csl].bitcast(F32R))
            for dt in range(MD):
                ps = pcp.tile([128, NB], F32, name="psC", tag="psC")
                nc.tensor.matmul(ps[:], dtw_t[dt][:], lr[:],
                                 start=True, stop=True)
                # softplus(x) = ln(1 + exp(x))
                ex = scp.tile([128, NB], BF16, name="ex", tag="ex")
                nc.scalar.activation(ex[:], ps[:], AF.Exp,
                                     bias=db_t[dt][:, 0:1])
                nc.scalar.activation(dl_t[(bb_, dt)][:, cloc], ex[:],
                                     AF.Ln, bias=1.0)
                nc.vector.tensor_tensor(du_t[(bb_, dt)][:, cloc],
                                        dl_t[(bb_, dt)][:, cloc],
                                        u_t[(bb_, dt)][:, cloc], OP.mult)


        for cm in reversed(pc_cms):
            cm.__exit__(None, None, None)

        # ---- phase C-scan + gating + out_proj, per batch ----
        ps_cms = [tc.tile_pool(name="woD", bufs=1),
                  tc.tile_pool(name="bcS", bufs=2),
                  tc.tile_pool(name="bcB", bufs=1),
                  tc.tile_pool(name="dAS", bufs=3),
                  tc.tile_pool(name="dBS", bufs=3),
                  tc.tile_pool(name="hS", bufs=1),
                  tc.tile_pool(name="hcS", bufs=1),
                  tc.tile_pool(name="gbS", bufs=1),
                  tc.tile_pool(name="psD", bufs=4, space="PSUM"),
                  tc.tile_pool(name="psX", bufs=4, space="PSUM"),
                  tc.tile_pool(name="stD", bufs=2)]
        wop, bsp, bbp, dap, dbp, hsp, hcp, gbp, pdp, pxp, sdp = \
            [cm.__enter__() for cm in ps_cms]

        wo_t = {}
        for kk in range(MD):
            for m in range(KH):
                wt = wop.tile([128, 128], BF16, name=f"wo{kk}_{m}",
                              tag=f"wo{kk}_{m}")
                nc.sync.dma_start(
                    wt[:], woT[kk * 128:(kk + 1) * 128, m * 128:(m + 1) * 128])
                wo_t[(kk, m)] = wt
        ones16 = wop.tile([1, 128], BF16, name="ones16", tag="ones16")
        nc.vector.memset(ones16[:], 1.0)

        NQ = L // NB
        seq = [(b, n) for b in range(B) for n in range(N)]
        prod = {}

        def produce(b, n):
            # replicate B/C rows across partitions via PE outer product;
            # rows staged at partition 0 (matmul base-partition constraint)
            rowB = bbp.tile([1, L], BF16, name="rowB", tag="rowB", bufs=1)
            nc.sync.dma_start(rowB[:], bc16_d[n:n + 1, b * L:(b + 1) * L])
            rowC = bbp.tile([1, L], BF16, name="rowC", tag="rowC", bufs=1)
            nc.sync.dma_start(rowC[:], bc16_d[N + n:N + n + 1,
                                              b * L:(b + 1) * L])
            Bt = bsp.tile([128, L], BF16, name="Bt", tag="Bt")
            Ct = bsp.tile([128, L], BF16, name="Ct", tag="Ct")
            for q in range(NQ):
                qsl = slice(q * NB, (q + 1) * NB)
                psq = pxp.tile([128, NB], F32, name="psX", tag="psX")
                nc.tensor.matmul(psq[:], ones16[:], rowB[:, qsl],
                                 start=True, stop=True)
                nc.scalar.copy(Bt[:, qsl], psq[:])
            for q in range(NQ):
                qsl = slice(q * NB, (q + 1) * NB)
                psq = pxp.tile([128, NB], F32, name="psX", tag="psX")
                nc.tensor.matmul(psq[:], ones16[:], rowC[:, qsl],
                                 start=True, stop=True)
                nc.scalar.copy(Ct[:, qsl], psq[:])
            return Bt, Ct

        def ensure(i):
            if 0 <= i < len(seq) and i not in prod:
                prod[i] = produce(*seq[i])

        ensure(0)
        ensure(1)
        for i, (b, n) in enumerate(seq):
            Bt, Ct = prod[i]
            bsl = slice(b * L, (b + 1) * L)
            if n == 0:
                # fresh bf16 accumulators for this batch (tag reuse across b);
                # ping-pong partner is the dead u tile
                for dt in range(MD):
                    y16[dt] = dp.tile([128, L], BF16, name=f"y16_{dt}",
                                      tag=f"y16_{dt}")
                    nc.vector.tensor_scalar_mul(
                        y16[dt][:], u_t[(b, dt)][:], D_t[dt][:, 0:1])
            if n == N - 1:
                gbs = {}
                for dt in range(2):
                    gb = gbp.tile([128, L], BF16, name="gb", tag="gb", bufs=2)
                    nc.sync.dma_start(
                        gb[:], gate_d[dt * 128:(dt + 1) * 128, bsl])
                    gbs[dt] = gb
            for dt in range(MD):
                dA = dap.tile([128, L], BF16, name="dA", tag="dA")
                nc.scalar.activation(dA[:], dl_t[(b, dt)][:], AF.Exp,
                                     scale=A_t[dt][:, n:n + 1])
                dBu = dbp.tile([128, L], BF16, name="dBu", tag="dBu")
                nc.gpsimd.tensor_tensor(dBu[:], du_t[(b, dt)][:], Bt[:],
                                        OP.mult)
                h = hsp.tile([128, L], BF16, name="h", tag="h")
                nc.vector.tensor_tensor_scan(h[:], dA[:], dBu[:], 0.0,
                                             op0=OP.mult, op1=OP.add)
                hc = hcp.tile([128, L], BF16, name="hc", tag="hc")
                nc.vector.tensor_tensor(hc[:], h[:], Ct[:], OP.mult)
                slot = [y16[dt][:], u_t[(b, dt)][:]]
                sj, dj = n % 2, 1 - (n % 2)
                if dt == MD - 1:
                    nc.gpsimd.tensor_tensor(slot[dj], slot[sj], hc[:], OP.add)
                else:
                    nc.vector.tensor_tensor(slot[dj], slot[sj], hc[:], OP.add)
            ensure(i + 2)
            if n != N - 1:
                continue
            # gating for this batch: write into the dead u tiles (bf16)
            ygs = []
            for dt in range(MD):
                if dt not in gbs:
                    gb = gbp.tile([128, L], BF16, name="gb", tag="gb", bufs=2)
                    nc.sync.dma_start(
                        gb[:], gate_d[dt * 128:(dt + 1) * 128, bsl])
                    gbs[dt] = gb
                nc.vector.tensor_tensor(u_t[(b, dt)][:], y16[dt][:],
                                        gbs[dt][:], OP.mult)
                ygs.append(u_t[(b, dt)])
            for m in range(KH):
                for cc in range(L // NB):
                    csl = slice(cc * NB, (cc + 1) * NB)
                    osl = slice(b * L + cc * NB, b * L + (cc + 1) * NB)
                    ps = pdp.tile([128, NB], F32, name="psD", tag="psD")
                    for kk in range(MD):
                        nc.tensor.matmul(ps[:], wo_t[(kk, m)][:],
                                         ygs[kk][:, csl],
                                         start=(kk == 0), stop=(kk == MD - 1))
                    st = sdp.tile([128, NB], BF16, name="stD", tag="stD")
                    nc.scalar.copy(st[:], ps[:])
                    nc.sync.dma_start(
                        out_part[m * 128:(m + 1) * 128, osl], st[:])

        for cm in reversed(ps_cms):
            cm.__exit__(None, None, None)
        dp_cm.__exit__(None, None, None)
        up_cm.__exit__(None, None, None)
        cp_cm.__exit__(None, None, None)

    _split_sync_waits(nc)
    return nc


def _bf(x):
    return np.ascontiguousarray(np.asarray(x, dtype=np.float32).astype(BF))


def make_in_maps(cfg, hidden_states, in_proj_w, conv_w, conv_b, x_proj_w,
                 dt_proj_w, dt_proj_b, A_log, D_param, out_proj_w):
    H, IL, N, R, B, L = cfg["H"], cfg["IL"], cfg["N"], cfg["R"], cfg["B"], cfg["L"]
    BL = B * L
    I_full = IL * N_CORES
    c = np.ascontiguousarray
    hsT = _bf(np.asarray(hidden_states, np.float32).reshape(BL, H).T)
    A_full = -np.exp(np.asarray(A_log, np.float32))
    in_maps = []
    for ci in range(N_CORES):
        sl = slice(ci * IL, (ci + 1) * IL)
        gsl = slice(I_full + ci * IL, I_full + (ci + 1) * IL)
        wxT = np.asarray(in_proj_w, np.float32)[sl, :].T
        wgT = np.asarray(in_proj_w, np.float32)[gsl, :].T
        in_maps.append({
            "hsT": hsT,
            "winT": _bf(np.concatenate([wxT, wgT], axis=1)),
            "convw": c(np.asarray(conv_w, np.float32)[sl, 0, :]),
            "convb": c(np.asarray(conv_b, np.float32)[sl].reshape(IL, 1)),
            "xwT": _bf(np.asarray(x_proj_w, np.float32)[:, sl].T),
            "dtwT": c(np.asarray(dt_proj_w, np.float32)[sl, :].T),
            "dtb": c(np.asarray(dt_proj_b, np.float32)[sl].reshape(IL, 1)),
            "Amat": c(A_full[sl, :]),
            "Dp": c(np.asarray(D_param, np.float32)[sl].reshape(IL, 1)),
            "woT": _bf(np.asarray(out_proj_w, np.float32)[:, sl].T),
        })
    return in_maps


_PROG_CACHE = {}


def run(cfg, inputs, **run_kwargs):
    key = tuple(sorted(cfg.items()))
    if key not in _PROG_CACHE:
        _PROG_CACHE[key] = build_program(cfg)
    nc = _PROG_CACHE[key]
    in_maps = make_in_maps(cfg, **inputs)
    res = run_bass_kernel_spmd(nc, in_maps, list(range(N_CORES)), **run_kwargs)
    H, B, L = cfg["H"], cfg["B"], cfg["L"]
    out = np.zeros((H, B * L), np.float64)
    for ci in range(N_CORES):
        out += np.asarray(res.results[ci]["out_part"], np.float32)
    full = out.astype(np.float32).T.reshape(B, L, H)
    return full, res


def kernel(**inputs):
    out, _ = run(CFG_FULL, inputs)
    return out
